# revision 13
# baseline (speedup 1.0000x reference)
"""Cubify kernel for Trainium2 (Bass/Tile), data-parallel over 8 NeuronCores.

Problem (hardcoded shapes): voxel_probas [8, 48, 48, 48] f32.
Reference outputs (tuple):
  face_mask [8, 6, 48, 48, 48] f32   -- occ & ~neighbor per direction
  faces     [6, 48, 48, 48, 2, 3] i32 -- input-independent constant
  vert_used [8, 117649] bool          -- vertex touched by any emitted face
  vert_pos  [117649, 3] f32           -- input-independent constant

Device algorithm (per core, one sample):
  Layout: grid rows r = z*48 + y packed 24 rows/partition -> partition
  p = 2*z + (y>=24), t = y%24, x in free dim.  All cross-partition data
  movement (z+-1 shifts, y seam rows, z vertex pool) goes through the
  TensorEngine with small constant matrices; a bidiagonal (I - shift)
  lhsT computes occ - neighbor in one accumulation, ACT Relu turns it
  into the face mask.  vert_used = 1 <= boxsum_2x2x2(occ_padded) <= 7,
  tested as |s8 - 4| < 3.5.
"""

import numpy as np
from contextlib import ExitStack

import ml_dtypes

import concourse.bass as bass
import concourse.tile as tile
from concourse import mybir
from concourse.bass_utils import run_bass_kernel_spmd

N, D, H, W = 8, 48, 48, 48
P = 96            # grid partitions: p = 2*z + (y // 24)
TPB = 24          # y-rows per partition
FD = TPB * W      # 1152 free elems/partition
VP = 98           # vertex partitions: 2*zc + h, zc in 0..48
VT = 25           # t' rows per vertex partition (yc = 24*h + t')
VX = 50           # padded xc extent (49 valid + 1 pad)
VFD = VT * VX     # 1250

_F32 = mybir.dt.float32
_BF16 = mybir.dt.bfloat16

# mats column offsets (all matrices stored as lhsT [K=96, M])
_OFF_I = 0        # identity [96,96]
_OFF_ZM = 96      # face z-: out[m] = occ[m] - occ[m-2]
_OFF_ZP = 192     # face z+: out[m] = occ[m] - occ[m+2]
_OFF_NEP = 288    # -1 superdiag on even cols (y+ seam)
_OFF_NEM = 384    # -1 subdiag on odd cols  (y- seam)
_OFF_S8 = 480     # z pool [96, 98]: out[m] = sy[m] + sy[m-2]
_OFF_BT0 = 578    # s8 t'=0 seam vs sx[:,23,:]  [96, 98]
_OFF_BT24 = 676   # s8 t'=24 seam vs sx[:,0,:]  [96, 98]
_MATS_COLS = 774

_cache: dict = {}


def _build_mats_f32() -> np.ndarray:
    m = np.zeros((P, _MATS_COLS), np.float32)
    m[:, _OFF_I:_OFF_I + P] = np.eye(P)
    zm = np.eye(P)
    for c in range(2, P):
        zm[c - 2, c] = -1.0
    m[:, _OFF_ZM:_OFF_ZM + P] = zm
    zp = np.eye(P)
    for c in range(P - 2):
        zp[c + 2, c] = -1.0
    m[:, _OFF_ZP:_OFF_ZP + P] = zp
    nep = np.zeros((P, P))
    for c in range(0, P, 2):
        nep[c + 1, c] = -1.0
    m[:, _OFF_NEP:_OFF_NEP + P] = nep
    nem = np.zeros((P, P))
    for c in range(1, P, 2):
        nem[c - 1, c] = -1.0
    m[:, _OFF_NEM:_OFF_NEM + P] = nem
    s8 = np.zeros((P, VP))
    for c in range(VP):
        if c <= P - 1:
            s8[c, c] = 1.0
        if c >= 2:
            s8[c - 2, c] = 1.0
    m[:, _OFF_S8:_OFF_S8 + VP] = s8
    bt0 = np.zeros((P, VP))
    for c in range(1, VP, 2):
        if c <= P - 1:
            bt0[c - 1, c] += 1.0
        if c >= 3:
            bt0[c - 3, c] += 1.0
    m[:, _OFF_BT0:_OFF_BT0 + VP] = bt0
    bt24 = np.zeros((P, VP))
    for c in range(0, VP, 2):
        if c <= P - 1:
            bt24[c + 1, c] += 1.0
        if c >= 2:
            bt24[c - 1, c] += 1.0
    m[:, _OFF_BT24:_OFF_BT24 + VP] = bt24
    return m


def _get_mats_bf16() -> np.ndarray:
    if "mats" not in _cache:
        _cache["mats"] = np.ascontiguousarray(
            _build_mats_f32().astype(ml_dtypes.bfloat16)
        )
    return _cache["mats"]


def _emit_body(ctx: ExitStack, tc, vox_d, mats_d, faces_d, vert_d):
    nc = tc.nc
    Alu = mybir.AluOpType
    Act = mybir.ActivationFunctionType

    sb = ctx.enter_context(tc.tile_pool(name="sb", bufs=1))
    pchunk = ctx.enter_context(tc.tile_pool(name="pchunk", bufs=3, space="PSUM"))
    pseam = ctx.enter_context(tc.tile_pool(name="pseam", bufs=2, space="PSUM"))
    ps8p = ctx.enter_context(tc.tile_pool(name="ps8p", bufs=1, space="PSUM"))

    # ---- inputs (vox split in halves so occ overlaps the load) ----
    vx = sb.tile([P, FD], _F32, name="vx")
    nc.sync.dma_start(out=vx[:, 0:FD // 2], in_=vox_d[:, 0:FD // 2])
    nc.sync.dma_start(out=vx[:, FD // 2:FD], in_=vox_d[:, FD // 2:FD])
    mats = sb.tile([P, _MATS_COLS], _BF16, name="mats")
    nc.sync.dma_start(out=mats, in_=mats_d[:, :])

    I96 = mats[:, _OFF_I:_OFF_I + P]
    Lzm = mats[:, _OFF_ZM:_OFF_ZM + P]
    Lzp = mats[:, _OFF_ZP:_OFF_ZP + P]
    nEp = mats[:, _OFF_NEP:_OFF_NEP + P]
    nEm = mats[:, _OFF_NEM:_OFF_NEM + P]
    Ls8 = mats[:, _OFF_S8:_OFF_S8 + VP]
    Bt0 = mats[:, _OFF_BT0:_OFF_BT0 + VP]
    Bt24 = mats[:, _OFF_BT24:_OFF_BT24 + VP]

    # ---- occupancy ----
    occ = sb.tile([P, TPB, W], _BF16, name="occ")
    occ_flat = occ.rearrange("p a b -> p (a b)")
    nc.vector.tensor_scalar(
        out=occ_flat[:, 0:FD // 2], in0=vx[:, 0:FD // 2], scalar1=0.5,
        scalar2=None, op0=Alu.is_gt,
    )
    nc.vector.tensor_scalar(
        out=occ_flat[:, FD // 2:FD], in0=vx[:, FD // 2:FD], scalar1=0.5,
        scalar2=None, op0=Alu.is_gt,
    )

    # ---- face tiles (f0=z-, f1=z+, f2=y-, f3=y+, f4=x-, f5=x+), paired so
    # each pair ships in ONE dma (adjacent in the DRAM layout) ----
    fzz = sb.tile([P, 2, TPB, W], _BF16, name="fzz")
    fyy = sb.tile([P, 2, TPB, W], _BF16, name="fyy")
    fxx = sb.tile([P, 2, TPB, W], _BF16, name="fxx")
    ft = [fzz[:, 0], fzz[:, 1], fyy[:, 0], fyy[:, 1], fxx[:, 0], fxx[:, 1]]
    ft_flat = [t.rearrange("p a b -> p (a b)") for t in ft]

    # ---- pool tiles ----
    sx = sb.tile([P, TPB, VX], _BF16, name="sx")     # x-pool, col 49 pad unused
    sy = sb.tile([P, VT - 2, VX], _BF16, name="sy")  # y-pool rows t'=1..23, col 49 = 0
    sy_flat = sy.rearrange("p a b -> p (a b)")       # [96, 1150]
    s8b = sb.tile([VP, VT, VX], _BF16, name="s8b")   # |boxsum - 4|
    s8b_flat = s8b.rearrange("p a b -> p (a b)")     # [98, 1250]
    vert = sb.tile([VP, VFD], _BF16, name="vert")

    # ---- GPSIMD stream: x-pool and y-pool (single-writer tiles) ----
    nc.gpsimd.tensor_tensor(
        out=sx[:, :, 1:48], in0=occ[:, :, 1:48], in1=occ[:, :, 0:47], op=Alu.add
    )
    nc.gpsimd.tensor_copy(out=sx[:, :, 0:1], in_=occ[:, :, 0:1])
    nc.gpsimd.tensor_copy(out=sx[:, :, 48:49], in_=occ[:, :, 47:48])
    nc.gpsimd.memset(sx[:, :, VX - 1:VX], 0.0)
    nc.gpsimd.memset(sy[:, :, VX - 1:VX], 0.0)
    nc.gpsimd.tensor_tensor(
        out=sy[:, :, 0:49], in0=sx[:, 1:TPB, 0:49], in1=sx[:, 0:TPB - 1, 0:49],
        op=Alu.add,
    )

    # ---- z faces: PE bidiagonal matmul + ACT relu, 512-col chunks ----
    zchunks = [(0, 512), (512, 1024), (1024, FD)]
    for lhsT, f in ((Lzm, 0), (Lzp, 1)):
        for c0, c1 in zchunks:
            pt = pchunk.tile([P, 512], _F32, name="pz", tag="chunk")
            nc.tensor.matmul(
                out=pt[:, : c1 - c0], lhsT=lhsT, rhs=occ_flat[:, c0:c1],
                start=True, stop=True,
            )
            nc.scalar.activation(
                out=ft_flat[f][:, c0:c1], in_=pt[:, : c1 - c0], func=Act.Relu
            )

    nc.sync.dma_start(
        out=faces_d[:, 0:2 * FD], in_=fzz.rearrange("p a c b -> p (a c b)")
    )

    # ---- x faces (DVE, free-dim shifts) ----
    nc.vector.tensor_tensor(
        out=ft[4][:, :, 1:48], in0=occ[:, :, 1:48], in1=occ[:, :, 0:47], op=Alu.is_gt
    )
    nc.vector.tensor_copy(out=ft[4][:, :, 0:1], in_=occ[:, :, 0:1])
    nc.vector.tensor_tensor(
        out=ft[5][:, :, 0:47], in0=occ[:, :, 0:47], in1=occ[:, :, 1:48], op=Alu.is_gt
    )
    nc.vector.tensor_copy(out=ft[5][:, :, 47:48], in_=occ[:, :, 47:48])
    nc.sync.dma_start(
        out=faces_d[:, 4 * FD:6 * FD], in_=fxx.rearrange("p a c b -> p (a c b)")
    )

    # ---- y faces: DVE main + PE/ACT seam rows ----
    # f2 = y-: rows t=1..23 in-partition; t=0 seam (even p: boundary occ; odd p: diff)
    nc.vector.tensor_tensor(
        out=ft[2][:, 1:TPB, :], in0=occ[:, 1:TPB, :], in1=occ[:, 0:TPB - 1, :],
        op=Alu.is_gt,
    )
    ps2 = pseam.tile([P, W], _F32, name="ps2", tag="seam")
    nc.tensor.matmul(out=ps2, lhsT=I96, rhs=occ[:, 0, :], start=True, stop=False)
    nc.tensor.matmul(out=ps2, lhsT=nEm, rhs=occ[:, TPB - 1, :], start=False, stop=True)
    nc.vector.tensor_scalar(
        out=ft[2][:, 0, :], in0=ps2, scalar1=0.0, scalar2=None, op0=Alu.max
    )
    # f3 = y+: rows t=0..22 in-partition; t=23 seam (even p: diff; odd p: boundary occ)
    nc.vector.tensor_tensor(
        out=ft[3][:, 0:TPB - 1, :], in0=occ[:, 0:TPB - 1, :], in1=occ[:, 1:TPB, :],
        op=Alu.is_gt,
    )
    ps3 = pseam.tile([P, W], _F32, name="ps3", tag="seam")
    nc.tensor.matmul(out=ps3, lhsT=I96, rhs=occ[:, TPB - 1, :], start=True, stop=False)
    nc.tensor.matmul(out=ps3, lhsT=nEp, rhs=occ[:, 0, :], start=False, stop=True)
    nc.vector.tensor_scalar(
        out=ft[3][:, TPB - 1, :], in0=ps3, scalar1=0.0, scalar2=None, op0=Alu.max
    )
    nc.sync.dma_start(
        out=faces_d[:, 2 * FD:4 * FD], in_=fyy.rearrange("p a c b -> p (a c b)")
    )

    # ---- vertex box-sum z-pool: one psum tile [98, 1250], 7 matmuls ----
    # free layout [t'=25, xc=50]; t'=0 row = cols 0..49, main rows = cols
    # 50..1200 (from sy rows 1..23), t'=24 row = cols 1200..1249.  Matmul
    # out slices stay within one 512-col psum bank each.
    ps8 = ps8p.tile([VP, VFD], _F32, name="ps8")
    # t'=0 seam: L_s8 @ sx[:,0,:] + B_t0 @ sx[:,23,:]
    nc.tensor.matmul(out=ps8[:, 0:50], lhsT=Ls8, rhs=sx[:, 0, 0:50],
                     start=True, stop=False)
    nc.tensor.matmul(out=ps8[:, 0:50], lhsT=Bt0, rhs=sx[:, TPB - 1, 0:50],
                     start=False, stop=True)
    # main rows: psum cols 50..1200 <- L_s8 @ sy cols 0..1150
    for pc0, pc1 in ((50, 512), (512, 1024), (1024, 1200)):
        nc.tensor.matmul(
            out=ps8[:, pc0:pc1], lhsT=Ls8, rhs=sy_flat[:, pc0 - 50:pc1 - 50],
            start=True, stop=True,
        )
    # t'=24 seam: L_s8 @ sx[:,23,:] + B_t24 @ sx[:,0,:]
    nc.tensor.matmul(out=ps8[:, 1200:1250], lhsT=Ls8, rhs=sx[:, TPB - 1, 0:50],
                     start=True, stop=False)
    nc.tensor.matmul(out=ps8[:, 1200:1250], lhsT=Bt24, rhs=sx[:, 0, 0:50],
                     start=False, stop=True)

    # ---- |s8-4| (ACT Abs, bias=-4 const AP) then <3.5 (DVE) ----
    nc.scalar.activation(out=s8b_flat, in_=ps8, func=Act.Abs, bias=-4.0)
    nc.vector.tensor_scalar(
        out=vert, in0=s8b_flat, scalar1=3.5, scalar2=None, op0=Alu.is_lt
    )
    nc.sync.dma_start(out=vert_d[:, :], in_=vert)


def _split_multi_waits(nc):
    """This walrus build rejects instructions carrying >1 sync wait; hoist
    extra waits onto single-wait drains inserted just before."""
    k = 0
    for bb in nc.m.functions[0].blocks:
        insts = bb.instructions
        i = 0
        while i < len(insts):
            ins = insts[i]
            si = ins.sync_info
            waits = list(si.on_wait) if si else []
            if len(waits) > 1:
                pre = []
                for w in waits[:-1]:
                    nd = mybir.InstDrain(name=f"I-waitsplit-{k}", ins=[], outs=[])
                    k += 1
                    nd.engine = ins.engine
                    nd.sync_info = mybir.SyncInfo(on_wait=[w], on_update=[])
                    pre.append(nd)
                ins.sync_info = mybir.SyncInfo(
                    on_wait=[waits[-1]], on_update=list(si.on_update)
                )
                insts[i:i] = pre
                i += len(pre)
            i += 1


def _get_module():
    if "nc" not in _cache:
        nc = bass.Bass(trn_type="TRN2")
        # register a -4.0 const AP (preamble, barrier'd) for the Abs bias
        _c4 = nc.alloc_sbuf_tensor("const-float32-m4", [128, 1], _F32)
        nc.gpsimd.memset(_c4.ap(), -4.0)
        nc.const_aps.aps[(_F32, -4.0)] = _c4.ap()
        nc.all_engine_barrier()
        vox_d = nc.dram_tensor("vox", [P, FD], _F32, kind="ExternalInput")
        mats_d = nc.dram_tensor("mats", [P, _MATS_COLS], _BF16, kind="ExternalInput")
        faces_d = nc.dram_tensor("faces", [P, 6 * FD], _BF16, kind="ExternalOutput")
        vert_d = nc.dram_tensor("vert", [VP, VFD], _BF16, kind="ExternalOutput")
        with tile.TileContext(nc) as tc:
            with ExitStack() as ctx:
                _emit_body(ctx, tc, vox_d, mats_d, faces_d, vert_d)
        _cache["nc"] = nc
    return _cache["nc"]


# ---------------- constant outputs (input-independent) ----------------

_CORNERS = np.array([
    [[0, 0, 0], [0, 0, 1], [0, 1, 0], [0, 1, 1]],  # z-1 face
    [[1, 0, 0], [1, 0, 1], [1, 1, 0], [1, 1, 1]],  # z+1 face
    [[1, 0, 0], [1, 0, 1], [0, 0, 0], [0, 0, 1]],  # y-1 face
    [[0, 1, 0], [0, 1, 1], [1, 1, 0], [1, 1, 1]],  # y+1 face
    [[1, 0, 0], [0, 0, 0], [1, 1, 0], [0, 1, 0]],  # x-1 face
    [[0, 0, 1], [1, 0, 1], [0, 1, 1], [1, 1, 1]],  # x+1 face
], dtype=np.int32)
_TRI = np.array([[0, 1, 2], [1, 2, 3]], dtype=np.int32)


def _const_outputs():
    if "faces_const" not in _cache:
        d, h, w = D, H, W
        zz, yy, xx = np.meshgrid(
            np.arange(d, dtype=np.int32), np.arange(h, dtype=np.int32),
            np.arange(w, dtype=np.int32), indexing="ij",
        )
        c = _CORNERS
        vz = zz[None, None] + c[:, :, 0, None, None, None]
        vy = yy[None, None] + c[:, :, 1, None, None, None]
        vx = xx[None, None] + c[:, :, 2, None, None, None]
        ids = (vz * (h + 1) + vy) * (w + 1) + vx            # [6,4,d,h,w]
        tri = ids[:, _TRI]                                   # [6,2,3,d,h,w]
        faces = np.ascontiguousarray(
            np.transpose(tri, (0, 3, 4, 5, 1, 2)).astype(np.int32)
        )
        V = (d + 1) * (h + 1) * (w + 1)
        g = np.arange(V, dtype=np.int32)
        vz2 = g // ((h + 1) * (w + 1))
        rem = g % ((h + 1) * (w + 1))
        vy2 = rem // (w + 1)
        vx2 = rem % (w + 1)
        vert_pos = np.stack([vz2, vy2, vx2], axis=-1).astype(np.float32) - 0.5
        _cache["faces_const"] = faces
        _cache["vert_pos_const"] = vert_pos
    return _cache["faces_const"], _cache["vert_pos_const"]


def _postprocess(results):
    face_mask = np.empty((N, 6, D, H, W), np.float32)
    vert_used = np.empty((N, (D + 1) * (H + 1) * (W + 1)), bool)
    for i, r in enumerate(results):
        fm = np.asarray(r["faces"]).astype(np.float32)          # [96, 6912]
        face_mask[i] = (
            fm.reshape(P, 6, TPB, W).transpose(1, 0, 2, 3).reshape(6, D, H, W)
        )
        v = np.asarray(r["vert"]).astype(np.float32).reshape(VP, VT, VX)[:, :, :49]
        vg = np.empty((49, 49, 49), np.float32)
        vg[:, 0:25] = v[0::2]
        vg[:, 24:49] = v[1::2]
        vert_used[i] = vg.reshape(-1) > 0.5
    return face_mask, vert_used


def _run(voxel_probas: np.ndarray, **kwargs):
    vox = np.ascontiguousarray(np.asarray(voxel_probas, dtype=np.float32))
    nc = _get_module()
    if not _cache.get("waits_split"):
        # walrus codegen rejects >1 sync wait per instruction; CoreSim's
        # simulate() harness dislikes the split drains, so apply just
        # before running on hardware.
        _split_multi_waits(nc)
        _cache["waits_split"] = True
    mats = _get_mats_bf16()
    in_maps = [
        {"vox": np.ascontiguousarray(vox[i].reshape(P, FD)), "mats": mats}
        for i in range(N)
    ]
    return run_bass_kernel_spmd(nc, in_maps, core_ids=list(range(N)), **kwargs)


def kernel(voxel_probas: np.ndarray):
    res = _run(voxel_probas)
    face_mask, vert_used = _postprocess(res.results)
    faces, vert_pos = _const_outputs()
    return face_mask, faces, vert_used, vert_pos
